# revision 17
# baseline (speedup 1.0000x reference)
"""Trainium2 Bass kernel for DecoderWithAttention (location-sensitive attention decoder).

Strategy: data-parallel over batch (64 -> 8 cores x 8), weights replicated.
One SPMD program; per-core data arrives via 3 DRAM input parameters:
  enc16 [8, 1024, 256] bf16, pk16 [128, PK] bf16 (weights/constants packed),
  pk32 [128, 1552] f32 (mask, exp bias, f32 identity, pre-masked one-hot).

Compute layout (per core, bf16 streams with f32 PSUM accumulation):
  x_fullT [128, 5*8]  x^T blocks: [e;1;pad | ctx0 | ctx1 | h2_0 | h2_1], col 8*xc+b
  gates   [8, 1024]   batch-major, 10 matmuls (x-chunks stationary, W moving)
  energy  32 psum chunks [128(a), 512(t)]: identity@enc_proj seed + stat13@aw13
          (dec enters via 8 one-hot contraction rows; conv via 5 shifted rows)
  ctx     aw transposed via PE, pre-masked one-hot cols (om8m), 64 matmuls
The energy seeds run 4 chunks ahead (rolling, cross-step) and the output MLP of
step t is emitted after gates(t+1) so the PE never idles during the pointwise.
h is stored doubled (h2 = 2h); consumer weights pre-scaled 0.5. Softmax uses a
constant shift vbound >= max(energy); masking is folded into om8m / sums.
"""

import numpy as np

V, E, D_ENC, H, A, NF, KW = 96, 64, 256, 256, 128, 10, 5
B, T_ENC, T_TGT = 64, 1024, 256
TD = T_TGT - 1            # 255 decoder steps
NCORES = 8
BL = B // NCORES          # 8 batch elements per core

# pk16 column layout
C_EYE = 0                 # [128, 128] identity (bf16)
C_WENC = 128              # [128, 2*128] W_enc^T chunks
C_WDEC = 384              # [128, 2*128] 0.5*W_dec^T chunks
C_W2 = 640                # [128, 2*96]  out_w2^T chunks
C_VOH = 832               # [128, 64]    v one-hot cols
C_ESTAT = 896             # [5, 128]     (W_loc @ conv)^T
C_ONES = 1024             # [1, 8] ones row
C_B2 = 1032               # [1, 96] out_b2
C_B1 = 1128               # [1, 256] out_b1
C_W1 = 1384               # [128, 4*256] out_w1^T chunks (ctx0 ctx1 h0 h1)
C_TOK = 2408              # [64, steps*8] token embeddings^T

# pk32 layout: 0:1024 mask (rows 0:8), 1024 nvb, 1026:1034 eye8, 1040:1552 om8m


def _pk_cols(steps):
    c_wg = C_TOK + steps * BL
    return c_wg, c_wg + 5 * 1024


_prog_cache = {}


def _build_program(steps=TD):
    import concourse.bass as bass
    import concourse.bacc as bacc
    import concourse.tile as tile
    from concourse import mybir
    from contextlib import ExitStack

    f32 = mybir.dt.float32
    bf16 = mybir.dt.bfloat16
    AF = mybir.ActivationFunctionType
    OP = mybir.AluOpType

    C_WG, PK_COLS = _pk_cols(steps)
    NCH = 16                  # 512-col energy chunks per step

    nc = bacc.Bacc(None, target_bir_lowering=False)

    enc_d = nc.declare_dram_parameter("enc16", [BL, T_ENC, D_ENC], bf16,
                                      isOutput=False)
    pk_d = nc.declare_dram_parameter("pk16", [128, PK_COLS], bf16, isOutput=False)
    p32_d = nc.declare_dram_parameter("pk32", [128, 1552], f32, isOutput=False)
    out_d = nc.declare_dram_parameter("out", [BL, steps, V], bf16, isOutput=True)

    with tile.TileContext(nc) as tc, ExitStack() as ctx:
        c1 = ctx.enter_context(tc.tile_pool(name="c1", bufs=1))

        # ---- persistent tiles ----
        pk = c1.tile([128, PK_COLS], bf16)
        p32 = c1.tile([128, 1552], f32)
        enc_m = c1.tile([128, BL, 8, D_ENC], bf16)     # [t%128, b, tc, d]
        enc_proj = c1.tile([128, BL * T_ENC], bf16)    # [a, b*1024+t]
        aw13 = c1.tile([13, BL * T_ENC], bf16)         # rows 0:8 onehot(b), 8:13 shifts
        stat13 = c1.tile([13, A], bf16)                # rows 0:8 dec, 8:13 estat
        awe32 = c1.tile([BL, T_ENC], f32)              # exp output (unmasked)
        awm32 = c1.tile([BL, T_ENC], f32)              # exp * mask (unnormalized)
        aw_n = c1.tile([BL, T_ENC + 4], bf16)          # normalized aw, zero halo
        awT8 = c1.tile([128, BL, 8, BL], bf16)         # [t%128, b, tc, j]
        x_fullT = c1.tile([128, 5 * BL], bf16)
        cT = c1.tile([BL, H], f32)                     # LSTM cell state
        tg = c1.tile([BL, H], f32)
        tc_s = c1.tile([BL, H], f32)
        scr_u = c1.tile([BL, H], f32)
        scr_w = c1.tile([BL, H], f32)
        h2 = c1.tile([BL, H], f32)
        hid_sb = c1.tile([BL, H], f32)
        hidT = c1.tile([128, 2 * BL], bf16)
        ctx_sb = c1.tile([BL, D_ENC], f32)
        sums = c1.tile([BL, 1], f32)
        rs = c1.tile([BL, 1], f32)

        # pk views
        eye16 = pk[:, C_EYE:C_EYE + 128]

        def wenc(dc):
            return pk[:, C_WENC + dc * 128:C_WENC + (dc + 1) * 128]

        def wdec(dc):
            return pk[:, C_WDEC + dc * 128:C_WDEC + (dc + 1) * 128]

        def w2(hc):
            return pk[:, C_W2 + hc * V:C_W2 + (hc + 1) * V]

        def voh(b):
            return pk[:, C_VOH + b * BL:C_VOH + (b + 1) * BL]

        ones_r = pk[0:1, C_ONES:C_ONES + BL]
        b2_r = pk[0:1, C_B2:C_B2 + V]
        b1_r = pk[0:1, C_B1:C_B1 + H]

        def w1T(xc):
            return pk[:, C_W1 + xc * H:C_W1 + (xc + 1) * H]

        def tok(t):
            return pk[0:64, C_TOK + t * BL:C_TOK + (t + 1) * BL]

        def wg(xc, half):
            o = C_WG + xc * 1024 + half * 512
            return pk[:, o:o + 512]

        mask32 = p32[0:BL, 0:T_ENC]
        nvb = p32[0:BL, 1024:1025]
        eye8 = p32[0:BL, 1026:1026 + BL]
        om8m = p32[:, 1040:1040 + 512]                 # [128, b*64+tc*8+j]

        # ---- const loads ----
        nc.sync.dma_start(out=pk, in_=pk_d[:])
        nc.sync.dma_start(out=p32, in_=p32_d[:])
        for b in range(BL):
            nc.sync.dma_start(
                out=enc_m[:, b, :, :],
                in_=enc_d[b].rearrange("(tc p) d -> p tc d", p=128),
            )
        nc.sync.dma_start(out=stat13[8:13, :], in_=pk[0:5, C_ESTAT:C_ESTAT + 128])

        # ---- state init ----
        nc.vector.memset(aw_n, 0.0)
        nc.vector.memset(aw_n[:, 2:2 + T_ENC], 1.0 / T_ENC)
        nc.vector.memset(x_fullT, 0.0)
        nc.vector.memset(x_fullT[64:65, 0:BL], 1.0)
        nc.vector.memset(cT, 0.0)
        nc.vector.memset(aw13, 0.0)
        onesrow = c1.tile([1, T_ENC], bf16)
        nc.vector.memset(onesrow, 1.0)
        for b in range(BL):
            # one-hot conv rows; DMA because DVE can't target partition b
            nc.sync.dma_start(out=aw13[b:b + 1, b * T_ENC:(b + 1) * T_ENC],
                              in_=onesrow)

        # ---- preamble: encT via DMA transposes, then enc_proj = W_enc @ enc^T ----
        with tc.tile_pool(name="pre", bufs=1) as pre, \
             tc.tile_pool(name="prep", bufs=2, space="PSUM") as prep:
            encT = pre.tile([128, 2, BL * T_ENC], bf16)   # [d%128, dc, b*1024+t]
            for b in range(BL):
                for tcb in range(8):
                    for dc in range(2):
                        eng = nc.sync if (tcb + dc) % 2 == 0 else nc.scalar
                        eng.dma_start(
                            out=encT[:, dc, b * T_ENC + tcb * 128:
                                     b * T_ENC + (tcb + 1) * 128],
                            in_=enc_m[:, b, tcb, dc * 128:(dc + 1) * 128],
                            transpose=True,
                        )
            for blk in range(BL * T_ENC // 512):
                ep_ps = prep.tile([128, 512], f32, tag="ep")
                for dc in range(2):
                    nc.tensor.matmul(
                        ep_ps[:, :], wenc(dc),
                        encT[:, dc, blk * 512:(blk + 1) * 512],
                        start=(dc == 0), stop=(dc == 1),
                    )
                nc.vector.tensor_copy(enc_proj[:, blk * 512:(blk + 1) * 512],
                                      ep_ps[:, :])

        # initial shift rows of aw13 from the uniform aw_n
        for k in range(KW):
            nc.sync.dma_start(out=aw13[8 + k:9 + k, :], in_=aw_n[:, k:k + T_ENC])

        # ---- psum pools ----
        pein = ctx.enter_context(tc.tile_pool(name="pein", bufs=2, space="PSUM"))
        pp = ctx.enter_context(tc.tile_pool(name="pp", bufs=2, space="PSUM"))
        pmm = ctx.enter_context(tc.tile_pool(name="pmm", bufs=2, space="PSUM"))
        tpool = ctx.enter_context(tc.tile_pool(name="tanh", bufs=3))
        lgpool = ctx.enter_context(tc.tile_pool(name="lg", bufs=3))

        def seed_chunk(b):
            """Open energy psum chunk b (cols b*1024) seeded with enc_proj."""
            ei = pein.tile([128, T_ENC], f32, tag="ei")
            for ch in range(2):
                nc.tensor.matmul(ei[:, ch * 512:(ch + 1) * 512], eye16,
                                 enc_proj[:, b * T_ENC + ch * 512:
                                          b * T_ENC + (ch + 1) * 512],
                                 start=True, stop=False)
            return ei

        def emit_tok_gates(t):
            nc.vector.tensor_copy(x_fullT[0:64, 0:BL], tok(t))
            gp_lo = pp.tile([BL, 512], f32, tag="pp")
            gp_hi = pp.tile([BL, 512], f32, tag="pp")
            for half, gp in ((0, gp_lo), (1, gp_hi)):
                for xc in range(5):
                    nc.tensor.matmul(
                        gp[:, :], x_fullT[:, xc * BL:(xc + 1) * BL],
                        wg(xc, half), start=(xc == 0), stop=(xc == 4),
                    )
            return gp_lo, gp_hi

        seed_q = [seed_chunk(b) for b in range(2)]
        gp_lo, gp_hi = emit_tok_gates(0)

        for t in range(steps):
            # (C) LSTM pointwise in [8, 256] space; i' = tanh(i/2) etc.
            nc.scalar.activation(gp_lo[:, :], gp_lo[:, :], AF.Tanh, scale=0.5)
            nc.scalar.activation(tg, gp_hi[:, H:2 * H], AF.Tanh)
            nc.scalar.activation(gp_hi[:, 0:H], gp_hi[:, 0:H], AF.Tanh, scale=0.5)
            nc.vector.scalar_tensor_tensor(scr_u, gp_lo[:, H:2 * H], 1.0, cT,
                                           OP.add, OP.mult)
            nc.vector.scalar_tensor_tensor(scr_w, gp_lo[:, 0:H], 1.0, tg,
                                           OP.add, OP.mult)
            nc.vector.tensor_tensor(scr_u, scr_u, scr_w, OP.add)
            nc.vector.tensor_scalar_mul(cT, scr_u, 0.5)
            nc.scalar.activation(tc_s, cT, AF.Tanh)
            nc.vector.scalar_tensor_tensor(h2, gp_hi[:, 0:H], 1.0, tc_s,
                                           OP.add, OP.mult)

            # (D) h2 -> x_fullT blocks 3,4; dec -> stat13 rows 0:8
            for hc in range(2):
                hT_ps = pmm.tile([128, BL], f32, tag="mm")
                nc.tensor.transpose(hT_ps[:, :], h2[:, hc * 128:(hc + 1) * 128],
                                    eye8)
                nc.vector.tensor_copy(x_fullT[:, (3 + hc) * BL:(4 + hc) * BL],
                                      hT_ps[:, :])
            dT_ps = pmm.tile([BL, A], f32, tag="mm")
            for dc in range(2):
                nc.tensor.matmul(dT_ps[:, :], x_fullT[:, (3 + dc) * BL:(4 + dc) * BL],
                                 wdec(dc), start=(dc == 0), stop=(dc == 1))
            nc.vector.tensor_copy(stat13[0:BL, :], dT_ps[:, :])

            # (E) energy: 8 chunks of [128, 1024], software-pipelined: voh runs
            # one chunk behind its tanh; a new seed reuses the buffer voh frees.
            en_lo = pp.tile([BL, 512], f32, tag="pp")
            en_hi = pp.tile([BL, 512], f32, tag="pp")
            th_q = []

            def emit_voh(b):
                th = th_q.pop(0)
                for ch, en in ((0, en_lo), (1, en_hi)):
                    nc.tensor.matmul(en[:, :], voh(b),
                                     th[:, ch * 512:(ch + 1) * 512],
                                     start=(b == 0), stop=(b == BL - 1))

            for b in range(BL):
                ei = seed_q.pop(0)
                for ch in range(2):
                    cols = slice(b * T_ENC + ch * 512, b * T_ENC + (ch + 1) * 512)
                    nc.tensor.matmul(ei[:, ch * 512:(ch + 1) * 512],
                                     stat13[:, :], aw13[:, cols],
                                     start=False, stop=True)
                th = tpool.tile([128, T_ENC], bf16, tag="th")
                nc.scalar.activation(th, ei[:, :], AF.Tanh)
                th_q.append(th)
                if b >= 1:
                    emit_voh(b - 1)
                    if b < BL - 1 or t + 1 < steps:
                        seed_q.append(seed_chunk((b + 1) % BL))
            emit_voh(BL - 1)
            if t + 1 < steps:
                seed_q.append(seed_chunk(1))

            # (F) softmax exp (constant shift); mask folded into om8m / sums
            nc.scalar.activation(awe32[:, 0:512], en_lo[:, :], AF.Exp, bias=nvb)
            nc.scalar.activation(awe32[:, 512:1024], en_hi[:, :], AF.Exp, bias=nvb)

            # (G)+(H) aw transpose + pre-masked one-hot + ctx, in two half-groups
            # so the first ctx matmuls start right after the first exp half
            cx_ps = pp.tile([BL, D_ENC], f32, tag="pp")
            for g in range(2):
                aT_g = pmm.tile([128, 4, BL], f32, tag="mm")
                for i in range(4):
                    tcb = g * 4 + i
                    nc.tensor.transpose(aT_g[:, i, :],
                                        awe32[:, tcb * 128:(tcb + 1) * 128], eye8)
                aT_bc = bass.AP(
                    tensor=aT_g[:, :, :].tensor, offset=aT_g[:, :, :].offset,
                    ap=[list(aT_g[:, :, :].ap[0]), [0, BL], [BL, 4], [1, BL]])
                om_g = bass.AP(
                    tensor=om8m.tensor, offset=om8m.offset + g * 32,
                    ap=[list(om8m.ap[0]), [64, BL], [8, 4], [1, BL]])
                nc.vector.tensor_tensor(awT8[:, :, g * 4:(g + 1) * 4, :],
                                        aT_bc, om_g, OP.mult)
                for i in range(4):
                    tcb = g * 4 + i
                    for b in range(BL):
                        nc.tensor.matmul(
                            cx_ps[:, :], awT8[:, b, tcb, :], enc_m[:, b, tcb, :],
                            start=(tcb == 0 and b == 0),
                            stop=(tcb == 7 and b == BL - 1),
                        )
            # sums/rs (needed by ctx-scale), then off-critical-path aw_n + shifts
            nc.vector.scalar_tensor_tensor(awm32, awe32, 1.0, mask32,
                                           OP.mult, OP.mult, accum_out=sums)
            nc.vector.reciprocal(rs, sums)
            nc.vector.tensor_scalar_mul(ctx_sb, cx_ps[:, :], rs[:, 0:1])
            nc.vector.tensor_scalar_mul(aw_n[:, 2:2 + T_ENC], awm32, rs[:, 0:1])
            for k in range(KW):
                eng = nc.sync if k % 2 == 0 else nc.gpsimd
                eng.dma_start(out=aw13[8 + k:9 + k, :],
                              in_=aw_n[:, k:k + T_ENC])

            # (I) ctx -> x_fullT blocks 1,2
            for hc in range(2):
                cT_ps = pmm.tile([128, BL], f32, tag="mm")
                nc.tensor.transpose(cT_ps[:, :], ctx_sb[:, hc * 128:(hc + 1) * 128],
                                    eye8)
                nc.vector.tensor_copy(x_fullT[:, (1 + hc) * BL:(2 + hc) * BL],
                                      cT_ps[:, :])

            # gates of the NEXT step first: the output MLP below then keeps the
            # PE busy while ACT/DVE run the next pointwise
            if t + 1 < steps:
                gp_lo, gp_hi = emit_tok_gates(t + 1)

            # (J) output MLP for step t
            hid_ps = pp.tile([BL, H], f32, tag="pp")
            for xc in range(4):
                nc.tensor.matmul(hid_ps[:, :], x_fullT[:, (1 + xc) * BL:(2 + xc) * BL],
                                 w1T(xc), start=(xc == 0), stop=False)
            nc.tensor.matmul(hid_ps[:, :], ones_r, b1_r, start=False, stop=True)
            nc.scalar.activation(hid_sb, hid_ps[:, :], AF.Tanh)
            for hc in range(2):
                hT_ps = pmm.tile([128, BL], f32, tag="mm")
                nc.tensor.transpose(hT_ps[:, :], hid_sb[:, hc * 128:(hc + 1) * 128],
                                    eye8)
                nc.vector.tensor_copy(hidT[:, hc * BL:(hc + 1) * BL], hT_ps[:, :])
            l_ps = pp.tile([BL, V], f32, tag="pp")
            for hc in range(2):
                nc.tensor.matmul(l_ps[:, :], hidT[:, hc * BL:(hc + 1) * BL],
                                 w2(hc), start=(hc == 0), stop=False)
            nc.tensor.matmul(l_ps[:, :], ones_r, b2_r, start=False, stop=True)
            lg = lgpool.tile([BL, V], bf16, tag="lg")
            nc.vector.tensor_copy(lg, l_ps[:, :])
            nc.gpsimd.dma_start(out=out_d[:, t, :], in_=lg)

    nc.compile()
    return nc


def _host_prep(inputs, core, steps=TD):
    """Build the per-core input map."""
    import ml_dtypes

    f = np.float32
    bf = ml_dtypes.bfloat16
    C_WG, PK_COLS = _pk_cols(steps)
    b0 = core * BL

    enc = np.asarray(inputs["encoder_outputs"][b0:b0 + BL], f)
    lengths = np.asarray(inputs["encoder_lengths"][b0:b0 + BL])
    targets = np.asarray(inputs["targets"][b0:b0 + BL])
    emb = np.asarray(inputs["emb"], f)
    W_ih = np.asarray(inputs["W_ih"], f)
    W_hh = np.asarray(inputs["W_hh"], f)
    bias = (np.asarray(inputs["b_ih"]) + np.asarray(inputs["b_hh"])).astype(f)
    conv_w = np.asarray(inputs["conv_w"], f)
    W_enc = np.asarray(inputs["W_enc"], f)
    W_dec = np.asarray(inputs["W_dec"], f)
    W_loc = np.asarray(inputs["W_loc"], f)
    v = np.asarray(inputs["v_w"], f)[0]
    out_w1 = np.asarray(inputs["out_w1"], f)
    out_b1 = np.asarray(inputs["out_b1"], f)
    out_w2 = np.asarray(inputs["out_w2"], f)
    out_b2 = np.asarray(inputs["out_b2"], f)

    pk = np.zeros((128, PK_COLS), bf)
    pk[:, C_EYE:C_EYE + 128] = np.eye(128, dtype=f)
    pk[:, C_WENC:C_WENC + 256] = W_enc.T.reshape(2, 128, A).transpose(1, 0, 2) \
        .reshape(128, 256)
    pk[:, C_WDEC:C_WDEC + 256] = (0.5 * W_dec.T).reshape(2, 128, A) \
        .transpose(1, 0, 2).reshape(128, 256)
    pk[:, C_W2:C_W2 + 2 * V] = out_w2.T.reshape(2, 128, V).transpose(1, 0, 2) \
        .reshape(128, 2 * V)
    vohm = np.zeros((A, BL * BL), f)
    for b in range(BL):
        vohm[:, b * BL + b] = v
    pk[:, C_VOH:C_VOH + 64] = vohm
    M = W_loc @ conv_w[:, 0, :]                     # [A, KW]
    pk[0:5, C_ESTAT:C_ESTAT + 128] = M.T
    pk[0, C_ONES:C_ONES + BL] = 1.0
    pk[0, C_B2:C_B2 + V] = out_b2
    pk[0, C_B1:C_B1 + H] = out_b1
    # w1 chunks: x blocks 1,2 = ctx, 3,4 = h2 (0.5-scaled)
    w1x = np.zeros((4, 128, H), f)
    w1x[0] = out_w1[:, H + 0:H + 128].T
    w1x[1] = out_w1[:, H + 128:H + 256].T
    w1x[2] = 0.5 * out_w1[:, 0:128].T
    w1x[3] = 0.5 * out_w1[:, 128:256].T
    pk[:, C_W1:C_W1 + 4 * H] = w1x.transpose(1, 0, 2).reshape(128, 4 * H)
    # token embeddings^T: col t*8+b
    tokT = emb[targets[:, :steps]]                  # [BL, steps, E]
    pk[0:64, C_TOK:C_TOK + steps * BL] = tokT.transpose(2, 1, 0).reshape(E, -1)
    # gate weights, gate order [i, f, o, g]; moving layout [128, 5, 1024]
    perm = np.concatenate([np.arange(0, 512), np.arange(768, 1024),
                           np.arange(512, 768)])
    Wg = np.concatenate([W_ih, W_hh], axis=1)[perm]      # [1024, 576]
    bias2 = bias[perm]
    Wg2 = np.zeros((640, 4 * H), f)
    Wg2[0:64] = Wg[:, 0:64].T
    Wg2[64] = bias2
    Wg2[128:384] = Wg[:, 64:320].T
    Wg2[384:640] = 0.5 * Wg[:, 320:576].T
    pk[:, C_WG:C_WG + 5 * 1024] = Wg2.reshape(5, 128, 1024).transpose(1, 0, 2) \
        .reshape(128, 5 * 1024)

    p32 = np.zeros((128, 1552), f)
    maskf = (np.arange(T_ENC)[None, :] < np.asarray(lengths)[:, None]).astype(f)
    p32[0:BL, 0:T_ENC] = maskf
    vbound = np.float32(np.abs(v).sum() + 1.0)
    p32[0:BL, 1024] = -vbound
    p32[0:BL, 1026:1026 + BL] = np.eye(BL, dtype=f)
    # om8m[p, b*64+tc*8+j] = (j==b) * mask[b, tc*128+p]
    om = np.zeros((128, BL, 8, BL), f)
    for b in range(BL):
        om[:, b, :, b] = maskf[b].reshape(8, 128).T
    p32[:, 1040:1552] = om.reshape(128, 512)

    return {
        "enc16": enc.astype(bf),
        "pk16": pk,
        "pk32": p32,
    }


def kernel(**inputs) -> np.ndarray:
    from concourse.bass_utils import run_bass_kernel_spmd

    if "prog" not in _prog_cache:
        _prog_cache["prog"] = _build_program()
    nc = _prog_cache["prog"]

    in_maps = [_host_prep(inputs, c) for c in range(NCORES)]
    res = run_bass_kernel_spmd(nc, in_maps, list(range(NCORES)))
    outs = [res.results[c]["out"].astype(np.float32) for c in range(NCORES)]
    return np.concatenate(outs, axis=0)


if __name__ == "__main__":
    import reference
    inputs = {k: np.asarray(v) for k, v in reference.setup_inputs().items()}
    got = kernel(**inputs)
    exp = np.asarray(reference.reference(**reference.setup_inputs()))
    err = np.abs(got - exp).max() / (np.abs(exp).max() + 1e-30)
    print("Relative error:", err)


# revision 22
# speedup vs baseline: 1.1495x; 1.1495x over previous
"""Trainium2 Bass kernel for DecoderWithAttention (location-sensitive attention decoder).

Strategy: data-parallel over batch (64 -> 8 cores x 8), weights replicated.
One SPMD program; per-core data arrives via 3 DRAM input parameters:
  enc16 [8, 1024, 256] bf16, pk16 [128, PK] bf16 (weights/constants packed),
  pk32 [128, 1552] f32 (mask, exp bias, f32 identity, pre-masked one-hot).

Compute layout (per core, bf16 streams with f32 PSUM accumulation):
  x_fullT [128, 5*8]  x^T blocks: [e;1;pad | ctx0 | ctx1 | h2_0 | h2_1], col 8*xc+b
  gates   [8, 1024]   batch-major, 10 matmuls (x-chunks stationary, W moving)
  energy  32 psum chunks [128(a), 512(t)]: identity@enc_proj seed + stat13@aw13
          (dec enters via 8 one-hot contraction rows; conv via 5 shifted rows)
  ctx     aw transposed via PE, pre-masked one-hot cols (om8m), 64 matmuls
The energy seeds run 4 chunks ahead (rolling, cross-step) and the output MLP of
step t is emitted after gates(t+1) so the PE never idles during the pointwise.
h is stored doubled (h2 = 2h); consumer weights pre-scaled 0.5. Softmax uses a
constant shift vbound >= max(energy); masking is folded into om8m / sums.
"""

import numpy as np

V, E, D_ENC, H, A, NF, KW = 96, 64, 256, 256, 128, 10, 5
B, T_ENC, T_TGT = 64, 1024, 256
TD = T_TGT - 1            # 255 decoder steps
NCORES = 8
BL = B // NCORES          # 8 batch elements per core

# pk16 column layout
C_EYE = 0                 # [128, 128] identity (bf16)
C_WENC = 128              # [128, 2*128] W_enc^T chunks
C_WDEC = 384              # [128, 2*128] 0.5*W_dec^T chunks
C_W2 = 640                # [128, 2*96]  out_w2^T chunks
C_VOH = 832               # [128, 64]    v one-hot cols
C_ESTAT = 896             # [5, 128]     (W_loc @ conv)^T
C_ONES = 1024             # [1, 8] ones row
C_B2 = 1032               # [1, 96] out_b2
C_B1 = 1128               # [1, 256] out_b1
C_W1 = 1384               # [128, 4*256] out_w1^T chunks (ctx0 ctx1 h0 h1)
C_TOK = 2408              # [64, steps*8] token embeddings^T

# pk32 layout: 0:1024 mask (rows 0:8), 1024 nvb, 1026:1034 eye8, 1040:1552 om8m


def _pk_cols(steps):
    c_wg = C_TOK + steps * BL
    return c_wg, c_wg + 5 * 1024


_prog_cache = {}


def _build_program(steps=TD):
    import concourse.bass as bass
    import concourse.bacc as bacc
    import concourse.tile as tile
    from concourse import mybir
    from contextlib import ExitStack

    f32 = mybir.dt.float32
    bf16 = mybir.dt.bfloat16
    AF = mybir.ActivationFunctionType
    OP = mybir.AluOpType

    C_WG, PK_COLS = _pk_cols(steps)
    NCH = 16                  # 512-col energy chunks per step

    nc = bacc.Bacc(None, target_bir_lowering=False)

    enc_d = nc.declare_dram_parameter("enc16", [BL, T_ENC, D_ENC], bf16,
                                      isOutput=False)
    pk_d = nc.declare_dram_parameter("pk16", [128, PK_COLS], bf16, isOutput=False)
    p32_d = nc.declare_dram_parameter("pk32", [128, 1552], f32, isOutput=False)
    out_d = nc.declare_dram_parameter("out", [BL, steps, V], bf16, isOutput=True)

    with tile.TileContext(nc) as tc, ExitStack() as ctx:
        c1 = ctx.enter_context(tc.tile_pool(name="c1", bufs=1))

        # ---- persistent tiles ----
        pk = c1.tile([128, PK_COLS], bf16)
        p32 = c1.tile([128, 1552], f32)
        enc_m = c1.tile([128, BL, 8, D_ENC], bf16)     # [t%128, b, tc, d]
        enc_proj = c1.tile([128, BL * T_ENC], f32)     # [a, b*1024+t]
        aw13 = c1.tile([13, BL * T_ENC], bf16)         # rows 0:8 onehot(b), 8:13 shifts
        stat13 = c1.tile([13, A], bf16)                # rows 0:8 dec, 8:13 estat
        awe32 = c1.tile([BL, T_ENC], f32)              # exp output (unmasked)
        awm32 = c1.tile([BL, T_ENC], f32)              # exp * mask (unnormalized)
        aw_n = c1.tile([BL, T_ENC + 4], bf16)          # normalized aw, zero halo
        awT8 = c1.tile([128, BL, 8, BL], bf16)         # [t%128, b, tc, j]
        x_fullT = c1.tile([128, 5 * BL], bf16)
        cT = c1.tile([BL, H], f32)                     # LSTM cell state
        tg = c1.tile([BL, H], f32)
        tc_s = c1.tile([BL, H], f32)
        scr_u = c1.tile([BL, H], f32)
        scr_w = c1.tile([BL, H], f32)
        h2 = c1.tile([BL, H], f32)
        hid_sb = c1.tile([BL, H], f32)
        hidT = c1.tile([128, 2 * BL], bf16)
        ctx_sb = c1.tile([BL, D_ENC], f32)
        sums = c1.tile([BL, 1], f32)
        rs = c1.tile([BL, 1], f32)

        # pk views
        eye16 = pk[:, C_EYE:C_EYE + 128]

        def wenc(dc):
            return pk[:, C_WENC + dc * 128:C_WENC + (dc + 1) * 128]

        def wdec(dc):
            return pk[:, C_WDEC + dc * 128:C_WDEC + (dc + 1) * 128]

        def w2(hc):
            return pk[:, C_W2 + hc * V:C_W2 + (hc + 1) * V]

        def voh(b):
            return pk[:, C_VOH + b * BL:C_VOH + (b + 1) * BL]

        ones_r = pk[0:1, C_ONES:C_ONES + BL]
        b2_r = pk[0:1, C_B2:C_B2 + V]
        b1_r = pk[0:1, C_B1:C_B1 + H]

        def w1T(xc):
            return pk[:, C_W1 + xc * H:C_W1 + (xc + 1) * H]

        def tok(t):
            return pk[0:64, C_TOK + t * BL:C_TOK + (t + 1) * BL]

        def wg(xc, half):
            o = C_WG + xc * 1024 + half * 512
            return pk[:, o:o + 512]

        mask32 = p32[0:BL, 0:T_ENC]
        nvb = p32[0:BL, 1024:1025]
        eye8 = p32[0:BL, 1026:1026 + BL]
        om8m = p32[:, 1040:1040 + 512]                 # [128, b*64+tc*8+j]

        # ---- const loads ----
        nc.sync.dma_start(out=pk, in_=pk_d[:])
        nc.sync.dma_start(out=p32, in_=p32_d[:])
        for b in range(BL):
            nc.sync.dma_start(
                out=enc_m[:, b, :, :],
                in_=enc_d[b].rearrange("(tc p) d -> p tc d", p=128),
            )
        nc.sync.dma_start(out=stat13[8:13, :], in_=pk[0:5, C_ESTAT:C_ESTAT + 128])

        # ---- state init ----
        nc.vector.memset(aw_n, 0.0)
        nc.vector.memset(aw_n[:, 2:2 + T_ENC], 1.0 / T_ENC)
        nc.vector.memset(x_fullT, 0.0)
        nc.vector.memset(x_fullT[64:65, 0:BL], 1.0)
        nc.vector.memset(cT, 0.0)
        nc.vector.memset(aw13, 0.0)
        onesrow = c1.tile([1, T_ENC], bf16)
        nc.vector.memset(onesrow, 1.0)
        for b in range(BL):
            # one-hot conv rows; DMA because DVE can't target partition b
            nc.sync.dma_start(out=aw13[b:b + 1, b * T_ENC:(b + 1) * T_ENC],
                              in_=onesrow)

        # ---- preamble: encT via DMA transposes, then enc_proj = W_enc @ enc^T ----
        with tc.tile_pool(name="pre", bufs=1) as pre, \
             tc.tile_pool(name="prep", bufs=2, space="PSUM") as prep:
            encT = pre.tile([128, 2, BL * T_ENC], bf16)   # [d%128, dc, b*1024+t]
            for b in range(BL):
                for tcb in range(8):
                    for dc in range(2):
                        eng = nc.sync if (tcb + dc) % 2 == 0 else nc.scalar
                        eng.dma_start(
                            out=encT[:, dc, b * T_ENC + tcb * 128:
                                     b * T_ENC + (tcb + 1) * 128],
                            in_=enc_m[:, b, tcb, dc * 128:(dc + 1) * 128],
                            transpose=True,
                        )
            for blk in range(BL * T_ENC // 512):
                ep_ps = prep.tile([128, 512], f32, tag="ep")
                for dc in range(2):
                    nc.tensor.matmul(
                        ep_ps[:, :], wenc(dc),
                        encT[:, dc, blk * 512:(blk + 1) * 512],
                        start=(dc == 0), stop=(dc == 1),
                    )
                nc.vector.tensor_copy(enc_proj[:, blk * 512:(blk + 1) * 512],
                                      ep_ps[:, :])

        # initial shift rows of aw13 from the uniform aw_n
        for k in range(KW):
            nc.sync.dma_start(out=aw13[8 + k:9 + k, :], in_=aw_n[:, k:k + T_ENC])

        # ---- psum pools ----
        pein = ctx.enter_context(tc.tile_pool(name="pein", bufs=2, space="PSUM"))
        pp = ctx.enter_context(tc.tile_pool(name="pp", bufs=2, space="PSUM"))
        pmm = ctx.enter_context(tc.tile_pool(name="pmm", bufs=2, space="PSUM"))
        tpool = ctx.enter_context(tc.tile_pool(name="tanh", bufs=3))
        lgpool = ctx.enter_context(tc.tile_pool(name="lg", bufs=3))

        def emit_tok_gates(t):
            nc.vector.tensor_copy(x_fullT[0:64, 0:BL], tok(t))
            gp_lo = pp.tile([BL, 512], f32, tag="pp")
            gp_hi = pp.tile([BL, 512], f32, tag="pp")
            for half, gp in ((0, gp_lo), (1, gp_hi)):
                for xc in range(5):
                    nc.tensor.matmul(
                        gp[:, :], x_fullT[:, xc * BL:(xc + 1) * BL],
                        wg(xc, half), start=(xc == 0), stop=(xc == 4),
                    )
            return gp_lo, gp_hi

        gp_lo, gp_hi = emit_tok_gates(0)

        for t in range(steps):
            # (C) LSTM pointwise in [8, 256] space; i' = tanh(i/2) etc.
            nc.scalar.activation(gp_lo[:, :], gp_lo[:, :], AF.Tanh, scale=0.5)
            nc.scalar.activation(tg, gp_hi[:, H:2 * H], AF.Tanh)
            nc.scalar.activation(gp_hi[:, 0:H], gp_hi[:, 0:H], AF.Tanh, scale=0.5)
            nc.vector.scalar_tensor_tensor(scr_u, gp_lo[:, H:2 * H], 1.0, cT,
                                           OP.add, OP.mult)
            nc.vector.scalar_tensor_tensor(scr_w, gp_lo[:, 0:H], 1.0, tg,
                                           OP.add, OP.mult)
            nc.vector.tensor_tensor(scr_u, scr_u, scr_w, OP.add)
            nc.vector.tensor_scalar_mul(cT, scr_u, 0.5)
            nc.scalar.activation(tc_s, cT, AF.Tanh)
            nc.vector.scalar_tensor_tensor(h2, gp_hi[:, 0:H], 1.0, tc_s,
                                           OP.add, OP.mult)

            # (D) h2 -> x_fullT blocks 3,4; dec -> stat13 rows 0:8
            for hc in range(2):
                hT_ps = pmm.tile([128, BL], f32, tag="mm")
                nc.tensor.transpose(hT_ps[:, :], h2[:, hc * 128:(hc + 1) * 128],
                                    eye8)
                nc.vector.tensor_copy(x_fullT[:, (3 + hc) * BL:(4 + hc) * BL],
                                      hT_ps[:, :])
            dT_ps = pmm.tile([BL, A], f32, tag="mm")
            for dc in range(2):
                nc.tensor.matmul(dT_ps[:, :], x_fullT[:, (3 + dc) * BL:(4 + dc) * BL],
                                 wdec(dc), start=(dc == 0), stop=(dc == 1))
            nc.vector.tensor_copy(stat13[0:BL, :], dT_ps[:, :])

            # (E) energy: 8 chunks. PSUM takes only loc+dec (estat); enc_proj
            # is added on the Vector engine on the way to SBUF; tanh in-place.
            # voh runs two chunks behind to hide the DVE+ACT latency.
            en_lo = pp.tile([BL, 512], f32, tag="pp")
            en_hi = pp.tile([BL, 512], f32, tag="pp")
            th_q = []

            def emit_voh(b):
                th = th_q.pop(0)
                for ch, en in ((0, en_lo), (1, en_hi)):
                    nc.tensor.matmul(en[:, :], voh(b),
                                     th[:, ch * 512:(ch + 1) * 512],
                                     start=(b == 0), stop=(b == BL - 1))

            for b in range(BL):
                ei = pein.tile([128, T_ENC], f32, tag="ei")
                for ch in range(2):
                    cols = slice(b * T_ENC + ch * 512, b * T_ENC + (ch + 1) * 512)
                    nc.tensor.matmul(ei[:, ch * 512:(ch + 1) * 512],
                                     stat13[:, :], aw13[:, cols],
                                     start=True, stop=True)
                th = tpool.tile([128, T_ENC], bf16, tag="th")
                nc.vector.tensor_tensor(th, ei[:, :],
                                        enc_proj[:, b * T_ENC:(b + 1) * T_ENC],
                                        OP.add)
                nc.scalar.activation(th, th, AF.Tanh)
                th_q.append(th)
                if b >= 2:
                    emit_voh(b - 2)
            emit_voh(BL - 2)
            emit_voh(BL - 1)

            # (F) softmax exp (constant shift); mask folded into om8m / sums
            nc.scalar.activation(awe32[:, 0:512], en_lo[:, :], AF.Exp, bias=nvb)
            nc.scalar.activation(awe32[:, 512:1024], en_hi[:, :], AF.Exp, bias=nvb)

            # (G)+(H) aw transpose + pre-masked one-hot + ctx, in two half-groups
            # so the first ctx matmuls start right after the first exp half
            cx_ps = pp.tile([BL, D_ENC], f32, tag="pp")
            for g in range(2):
                aT_g = pmm.tile([128, 4, BL], f32, tag="mm")
                for i in range(4):
                    tcb = g * 4 + i
                    nc.tensor.transpose(aT_g[:, i, :],
                                        awe32[:, tcb * 128:(tcb + 1) * 128], eye8)
                aT_bc = bass.AP(
                    tensor=aT_g[:, :, :].tensor, offset=aT_g[:, :, :].offset,
                    ap=[list(aT_g[:, :, :].ap[0]), [0, BL], [BL, 4], [1, BL]])
                om_g = bass.AP(
                    tensor=om8m.tensor, offset=om8m.offset + g * 32,
                    ap=[list(om8m.ap[0]), [64, BL], [8, 4], [1, BL]])
                nc.vector.tensor_tensor(awT8[:, :, g * 4:(g + 1) * 4, :],
                                        aT_bc, om_g, OP.mult)
                for i in range(4):
                    tcb = g * 4 + i
                    for b in range(BL):
                        nc.tensor.matmul(
                            cx_ps[:, :], awT8[:, b, tcb, :], enc_m[:, b, tcb, :],
                            start=(tcb == 0 and b == 0),
                            stop=(tcb == 7 and b == BL - 1),
                        )
            # sums/rs (needed by ctx-scale), then off-critical-path aw_n + shifts
            nc.vector.scalar_tensor_tensor(awm32, awe32, 1.0, mask32,
                                           OP.mult, OP.mult, accum_out=sums)
            nc.vector.reciprocal(rs, sums)
            nc.vector.tensor_scalar_mul(ctx_sb, cx_ps[:, :], rs[:, 0:1])
            nc.vector.tensor_scalar_mul(aw_n[:, 2:2 + T_ENC], awm32, rs[:, 0:1])
            for k in range(KW):
                eng = nc.sync if k % 2 == 0 else nc.gpsimd
                eng.dma_start(out=aw13[8 + k:9 + k, :],
                              in_=aw_n[:, k:k + T_ENC])

            # (I) ctx -> x_fullT blocks 1,2
            for hc in range(2):
                cT_ps = pmm.tile([128, BL], f32, tag="mm")
                nc.tensor.transpose(cT_ps[:, :], ctx_sb[:, hc * 128:(hc + 1) * 128],
                                    eye8)
                nc.vector.tensor_copy(x_fullT[:, (1 + hc) * BL:(2 + hc) * BL],
                                      cT_ps[:, :])

            # gates of the NEXT step first: the output MLP below then keeps the
            # PE busy while ACT/DVE run the next pointwise
            if t + 1 < steps:
                gp_lo, gp_hi = emit_tok_gates(t + 1)

            # (J) output MLP for step t
            hid_ps = pp.tile([BL, H], f32, tag="pp")
            for xc in range(4):
                nc.tensor.matmul(hid_ps[:, :], x_fullT[:, (1 + xc) * BL:(2 + xc) * BL],
                                 w1T(xc), start=(xc == 0), stop=False)
            nc.tensor.matmul(hid_ps[:, :], ones_r, b1_r, start=False, stop=True)
            nc.scalar.activation(hid_sb, hid_ps[:, :], AF.Tanh)
            for hc in range(2):
                hT_ps = pmm.tile([128, BL], f32, tag="mm")
                nc.tensor.transpose(hT_ps[:, :], hid_sb[:, hc * 128:(hc + 1) * 128],
                                    eye8)
                nc.vector.tensor_copy(hidT[:, hc * BL:(hc + 1) * BL], hT_ps[:, :])
            l_ps = pp.tile([BL, V], f32, tag="pp")
            for hc in range(2):
                nc.tensor.matmul(l_ps[:, :], hidT[:, hc * BL:(hc + 1) * BL],
                                 w2(hc), start=(hc == 0), stop=False)
            nc.tensor.matmul(l_ps[:, :], ones_r, b2_r, start=False, stop=True)
            lg = lgpool.tile([BL, V], bf16, tag="lg")
            nc.vector.tensor_copy(lg, l_ps[:, :])
            nc.gpsimd.dma_start(out=out_d[:, t, :], in_=lg)

    nc.compile()
    return nc


def _host_prep(inputs, core, steps=TD):
    """Build the per-core input map."""
    import ml_dtypes

    f = np.float32
    bf = ml_dtypes.bfloat16
    C_WG, PK_COLS = _pk_cols(steps)
    b0 = core * BL

    enc = np.asarray(inputs["encoder_outputs"][b0:b0 + BL], f)
    lengths = np.asarray(inputs["encoder_lengths"][b0:b0 + BL])
    targets = np.asarray(inputs["targets"][b0:b0 + BL])
    emb = np.asarray(inputs["emb"], f)
    W_ih = np.asarray(inputs["W_ih"], f)
    W_hh = np.asarray(inputs["W_hh"], f)
    bias = (np.asarray(inputs["b_ih"]) + np.asarray(inputs["b_hh"])).astype(f)
    conv_w = np.asarray(inputs["conv_w"], f)
    W_enc = np.asarray(inputs["W_enc"], f)
    W_dec = np.asarray(inputs["W_dec"], f)
    W_loc = np.asarray(inputs["W_loc"], f)
    v = np.asarray(inputs["v_w"], f)[0]
    out_w1 = np.asarray(inputs["out_w1"], f)
    out_b1 = np.asarray(inputs["out_b1"], f)
    out_w2 = np.asarray(inputs["out_w2"], f)
    out_b2 = np.asarray(inputs["out_b2"], f)

    pk = np.zeros((128, PK_COLS), bf)
    pk[:, C_EYE:C_EYE + 128] = np.eye(128, dtype=f)
    pk[:, C_WENC:C_WENC + 256] = W_enc.T.reshape(2, 128, A).transpose(1, 0, 2) \
        .reshape(128, 256)
    pk[:, C_WDEC:C_WDEC + 256] = (0.5 * W_dec.T).reshape(2, 128, A) \
        .transpose(1, 0, 2).reshape(128, 256)
    pk[:, C_W2:C_W2 + 2 * V] = out_w2.T.reshape(2, 128, V).transpose(1, 0, 2) \
        .reshape(128, 2 * V)
    vohm = np.zeros((A, BL * BL), f)
    for b in range(BL):
        vohm[:, b * BL + b] = v
    pk[:, C_VOH:C_VOH + 64] = vohm
    M = W_loc @ conv_w[:, 0, :]                     # [A, KW]
    pk[0:5, C_ESTAT:C_ESTAT + 128] = M.T
    pk[0, C_ONES:C_ONES + BL] = 1.0
    pk[0, C_B2:C_B2 + V] = out_b2
    pk[0, C_B1:C_B1 + H] = out_b1
    # w1 chunks: x blocks 1,2 = ctx, 3,4 = h2 (0.5-scaled)
    w1x = np.zeros((4, 128, H), f)
    w1x[0] = out_w1[:, H + 0:H + 128].T
    w1x[1] = out_w1[:, H + 128:H + 256].T
    w1x[2] = 0.5 * out_w1[:, 0:128].T
    w1x[3] = 0.5 * out_w1[:, 128:256].T
    pk[:, C_W1:C_W1 + 4 * H] = w1x.transpose(1, 0, 2).reshape(128, 4 * H)
    # token embeddings^T: col t*8+b
    tokT = emb[targets[:, :steps]]                  # [BL, steps, E]
    pk[0:64, C_TOK:C_TOK + steps * BL] = tokT.transpose(2, 1, 0).reshape(E, -1)
    # gate weights, gate order [i, f, o, g]; moving layout [128, 5, 1024]
    perm = np.concatenate([np.arange(0, 512), np.arange(768, 1024),
                           np.arange(512, 768)])
    Wg = np.concatenate([W_ih, W_hh], axis=1)[perm]      # [1024, 576]
    bias2 = bias[perm]
    Wg2 = np.zeros((640, 4 * H), f)
    Wg2[0:64] = Wg[:, 0:64].T
    Wg2[64] = bias2
    Wg2[128:384] = Wg[:, 64:320].T
    Wg2[384:640] = 0.5 * Wg[:, 320:576].T
    pk[:, C_WG:C_WG + 5 * 1024] = Wg2.reshape(5, 128, 1024).transpose(1, 0, 2) \
        .reshape(128, 5 * 1024)

    p32 = np.zeros((128, 1552), f)
    maskf = (np.arange(T_ENC)[None, :] < np.asarray(lengths)[:, None]).astype(f)
    p32[0:BL, 0:T_ENC] = maskf
    vbound = np.float32(np.abs(v).sum() + 1.0)
    p32[0:BL, 1024] = -vbound
    p32[0:BL, 1026:1026 + BL] = np.eye(BL, dtype=f)
    # om8m[p, b*64+tc*8+j] = (j==b) * mask[b, tc*128+p]
    om = np.zeros((128, BL, 8, BL), f)
    for b in range(BL):
        om[:, b, :, b] = maskf[b].reshape(8, 128).T
    p32[:, 1040:1552] = om.reshape(128, 512)

    return {
        "enc16": enc.astype(bf),
        "pk16": pk,
        "pk32": p32,
    }


def kernel(**inputs) -> np.ndarray:
    from concourse.bass_utils import run_bass_kernel_spmd

    if "prog" not in _prog_cache:
        _prog_cache["prog"] = _build_program()
    nc = _prog_cache["prog"]

    in_maps = [_host_prep(inputs, c) for c in range(NCORES)]
    res = run_bass_kernel_spmd(nc, in_maps, list(range(NCORES)))
    outs = [res.results[c]["out"].astype(np.float32) for c in range(NCORES)]
    return np.concatenate(outs, axis=0)


if __name__ == "__main__":
    import reference
    inputs = {k: np.asarray(v) for k, v in reference.setup_inputs().items()}
    got = kernel(**inputs)
    exp = np.asarray(reference.reference(**reference.setup_inputs()))
    err = np.abs(got - exp).max() / (np.abs(exp).max() + 1e-30)
    print("Relative error:", err)
